# revision 2
# baseline (speedup 1.0000x reference)
"""Multi-head attention Bass/Tile kernel for Trainium2, 8 cores data-parallel.

Shapes (hardcoded): x [8, 1024, 768], Wqkv [768, 2304], bqkv [2304],
Wproj [768, 768], bproj [768].  B=8 batches -> one batch per NeuronCore.

v2 dataflow per core (v/out path fp16, q/k path fp8 DoubleRow):
  v     [n, c'] : stationary = xT-tiles fp16, moving = Wv_aug (c' = 8*97,
                  ones col per head for the softmax denominator)
  qT/kT [96, n] : fp8 DoubleRow, K=256 per matmul (3 per 512-chunk);
                  W pre-scaled by 64 host-side (fp8 subnormal range), the
                  64*64 factor is divided out of the exp scale.  q bias is
                  added by DVE tensor_scalar_add on the psum->sbuf copy.
  S^T   [j, i]  : psum [128, 1024] (2 banks), 2 MMs, K=96
  expS^T        : one ACT exp per (h, j) tile with fused *E^-0.5/4096 scale;
                  ACT runs nothing but exp
  o_aug^T [d,i] : stationary = v head cols (96 + ones), moving = expS^T
                  -> row 96 = softmax denominator (colsum)
  normalize     : DVE recip of the denom row; gpsimd partition_broadcast
                  replicates it (no DRAM bounce); DVE fp16 multiply; then
                  SBUF->SBUF DMA repacks the 96-row head block into
                  128-row o_all tiles for a K=128 output projection
  out   [i, e]  : 6 MMs K=128 per (i, chunk); fp16 out, DMA to DRAM;
                  proj+v biases folded on host.
"""

import numpy as np
import ml_dtypes

import concourse.bass as bass
import concourse.bacc as bacc
import concourse.mybir as mybir
import concourse.tile as tile

B, N, E, H = 8, 1024, 768, 8
D = E // H          # 96
DA = D + 1          # 97: head dim + ones column for colsum
NT = N // 128       # 8 token tiles
ET = E // 128       # 6 embedding k-tiles
KP = E // 256       # 3 fp8 DoubleRow k-pair tiles
WS = 64.0           # host-side Wq/Wk scale (fp8 subnormal dodge)
SCALE = float(E) ** -0.5 / (WS * WS)

F16 = mybir.dt.float16
F32 = mybir.dt.float32
F8 = mybir.dt.float8e4
NP8 = ml_dtypes.float8_e4m3
DR = mybir.MatmulPerfMode.DoubleRow
EXP = mybir.ActivationFunctionType.Exp
COPY = mybir.ActivationFunctionType.Copy


def build_program(repeats=1, loop_n=0):
    """loop_n > 0 wraps the body in a hardware For_i loop (timing use)."""
    import contextlib
    nc = bacc.Bacc("TRN2", target_bir_lowering=False)

    xT = nc.dram_tensor("xT", [E, N], F16, kind="ExternalInput")
    x8 = nc.dram_tensor("x8", [KP, 128, 2, N], F8, kind="ExternalInput")
    wq8 = nc.dram_tensor("wq8", [KP, 128, 2, E], F8, kind="ExternalInput")
    wk8 = nc.dram_tensor("wk8", [KP, 128, 2, E], F8, kind="ExternalInput")
    qb = nc.dram_tensor("qb", [D, H], F32, kind="ExternalInput")
    wv = nc.dram_tensor("wv", [E, H * DA], F16, kind="ExternalInput")
    wp = nc.dram_tensor("wp", [E, E], F16, kind="ExternalInput")
    out = nc.dram_tensor("out", [N, E], F16, kind="ExternalOutput")

    with tile.TileContext(nc) as tc:
        with (
            tc.tile_pool(name="persist", bufs=1) as persist,
            tc.tile_pool(name="exps", bufs=2) as exps,
            tc.tile_pool(name="osb", bufs=2) as osb,
            tc.tile_pool(name="outsb", bufs=8) as outp,
            tc.tile_pool(name="mmps", bufs=2, space="PSUM") as mmps,
            tc.tile_pool(name="stps", bufs=2, space="PSUM") as stps,
            tc.tile_pool(name="avps", bufs=2, space="PSUM") as avps,
            tc.tile_pool(name="dramp", bufs=2, space="DRAM") as dramp,
        ):
            loop_cm = (tc.For_i(0, loop_n, 1,
                                hint_engines=tuple(mybir.ALL_ENGINES))
                       if loop_n > 0 else contextlib.nullcontext())
            with loop_cm:
             for _rep in range(repeats):
                # ---------------- load inputs ----------------
                # DMA order = first-use order: x+wv (v phase) first, then the
                # fp8 q/k operands, then wp
                x_sb, wv_sb, x8_sb, wq8_sb, wk8_sb = [], [], [], [], []
                # load order = first-use order: the fp8 q/k operands feed the
                # first PE work (qk(0) -> S^T(0)); x/wv stream behind them for
                # the v-proj filler groups in the head-0/1 slots.  Two HWDGE
                # queues (SP + ACT) run in parallel.
                qb_sb = persist.tile([D, H], F32, tag="qb", name="qb")
                nc.sync.dma_start(out=qb_sb, in_=qb[:, :])
                for t in range(KP):
                    xt = persist.tile([128, 2, N], F8, tag=f"x8{t}", name=f"x8{t}")
                    nc.scalar.dma_start(out=xt[:, :, 0:512],
                                        in_=x8[t, :, :, 0:512])
                    x8_sb.append(xt)
                    qt = persist.tile([128, 2, E], F8, tag=f"wq8{t}", name=f"wq8{t}")
                    nc.sync.dma_start(out=qt, in_=wq8[t, :, :, :])
                    wq8_sb.append(qt)
                for t in range(KP):
                    nc.scalar.dma_start(out=x8_sb[t][:, :, 512:N],
                                        in_=x8[t, :, :, 512:N])
                    kt = persist.tile([128, 2, E], F8, tag=f"wk8{t}", name=f"wk8{t}")
                    nc.sync.dma_start(out=kt, in_=wk8[t, :, :, :])
                    wk8_sb.append(kt)
                for k in range(ET):
                    xk = persist.tile([128, N], F16, tag=f"x{k}", name=f"x{k}")
                    nc.scalar.dma_start(out=xk[:, 0:512],
                                        in_=xT[k * 128:(k + 1) * 128, 0:512])
                    x_sb.append(xk)
                    vk = persist.tile([128, H * DA], F16, tag=f"wv{k}", name=f"wv{k}")
                    nc.sync.dma_start(out=vk[:, 0:512],
                                      in_=wv[k * 128:(k + 1) * 128, 0:512])
                    wv_sb.append(vk)
                for k in range(ET):
                    nc.sync.dma_start(out=wv_sb[k][:, 512:H * DA],
                                      in_=wv[k * 128:(k + 1) * 128, 512:H * DA])
                for k in range(ET):
                    nc.scalar.dma_start(out=x_sb[k][:, 512:N],
                                        in_=xT[k * 128:(k + 1) * 128, 512:N])
                wp_sb = []
                for t in range(ET):
                    pt = persist.tile([128, E], F16, tag=f"wp{t}", name=f"wp{t}")
                    eng = nc.sync if t % 2 == 0 else nc.scalar
                    eng.dma_start(out=pt, in_=wp[t * 128:(t + 1) * 128, :])
                    wp_sb.append(pt)

                # ---------------- QKV projections ----------------
                qT = [persist.tile([D, N], F16, tag=f"qT{c}", name=f"qT{c}")
                      for c in range(H)]
                kT = [persist.tile([D, N], F16, tag=f"kT{c}", name=f"kT{c}")
                      for c in range(H)]
                v_sb = [persist.tile([128, H * DA], F16, tag=f"v{n}", name=f"v{n}")
                        for n in range(NT)]

                # v groups are emitted as PE filler inside head-0/1
                # slots (the exp stream leaves the PE half idle there);
                # group order follows DMA arrival
                def emit_v(n, off, w):
                    ns = slice(n * 128, (n + 1) * 128)
                    ps = mmps.tile([128, w], F32, tag="mm", name="ps_v")
                    for k in range(ET):
                        nc.tensor.matmul(
                            ps, x_sb[k][:, ns], wv_sb[k][:, off:off + w],
                            start=(k == 0), stop=(k == ET - 1))
                    nc.vector.tensor_copy(v_sb[n][:, off:off + w], ps)
                    if off == 512:
                        # ones column per head (colsum trick); softmax makes
                        # the k-bias terms cancel, the v-bias folds on host.
                        # Emitted after the second chunk copy of this n-tile
                        # so the copies don't clobber it.
                        nc.gpsimd.memset(
                            v_sb[n].rearrange("p (h a) -> p h a", h=H)[:, :, D],
                            1.0)

                W2 = H * DA - 512
                vgroups = ([(n, 0, 512) for n in range(4)]
                           + [(n, 512, W2) for n in range(4)]
                           + [(n, 0, 512) for n in range(4, NT)]
                           + [(n, 512, W2) for n in range(4, NT)])

                # emit_qk_group(h, idx): one fp8 DoubleRow psum accumulation
                # group (idx 0/1 = q chunks, 2/3 = k chunks)
                def emit_qk_group(h, idx):
                    w8, dst = (wq8_sb, qT[h]) if idx < 2 else (wk8_sb, kT[h])
                    cs = slice(h * D, (h + 1) * D)
                    off = (idx % 2) * 512
                    ps = mmps.tile([D, 512], F32, tag="mm", name="ps_qk")
                    for t in range(KP):
                        nc.tensor.matmul(
                            ps, w8[t][:, :, cs], x8_sb[t][:, :, off:off + 512],
                            start=(t == 0), stop=(t == KP - 1), perf_mode=DR)
                    if idx < 2:
                        nc.vector.tensor_scalar_add(
                            dst[:, off:off + 512], ps, qb_sb[:, h:h + 1])
                    else:
                        nc.vector.tensor_copy(dst[:, off:off + 512], ps)

                o_all = [persist.tile([128, N], F16, tag=f"oa{t}", name=f"oa{t}")
                         for t in range(ET)]

                def emit_av(h, ex, off):
                    hs = slice(h * DA, (h + 1) * DA)
                    av = avps.tile([DA, 512], F32, tag="av", bufs=2,
                                   name="av_ps")
                    for j in range(NT):
                        nc.tensor.matmul(
                            av, v_sb[j][:, hs], ex[j][:, off:off + 512],
                            start=(j == 0), stop=(j == NT - 1))
                    nc.vector.tensor_copy(o_sb[h][:, off:off + 512], av)

                def emit_norm(h, step):
                    if step == 0:
                        # reciprocal of the denominator row, then replicate it
                        # across partitions on the (idle) gpsimd engine
                        rcp[h] = osb.tile([1, N], F16, tag="rcp",
                                          name=f"rcp{h}")
                        with nc.allow_low_precision(reason="denom ~1e3"):
                            nc.vector.reciprocal(rcp[h], o_sb[h][D:DA, :])
                        rbc[h] = osb.tile([D, N], F16, tag="rbc",
                                          name=f"rbc{h}")
                        # replicate across partitions with a DRAM bounce: the
                        # DMA engines are free, and gpsimd's SBUF port is
                        # shared with DVE (measured slower)
                        dr = dramp.tile([1, N], F16, tag="drcp",
                                        name=f"drcp{h}")
                        nc.sync.dma_start(out=dr[0:1, :], in_=rcp[h][0:1, :])
                        bc = bass.AP(
                            tensor=dr.tensor, offset=dr.offset,
                            ap=[[0, D]] + [list(d) for d in dr[0:1, :].ap[1:]])
                        nc.sync.dma_start(out=rbc[h], in_=bc)
                    else:
                        onr = osb.tile([D, N], F16, tag="onr", name=f"onr{h}")
                        nc.vector.tensor_mul(onr, o_sb[h][0:D, :], rbc[h])
                        # repack the 96-row head block into 128-row o_all
                        # tiles (SBUF->SBUF DMA shifts partitions) so the
                        # output projection contracts K=128 per matmul
                        r0 = D * h
                        t0, p0 = r0 // 128, r0 % 128
                        n1 = min(D, 128 - p0)
                        nc.sync.dma_start(out=o_all[t0][p0:p0 + n1, :],
                                          in_=onr[0:n1, :])
                        if n1 < D:
                            nc.sync.dma_start(out=o_all[t0 + 1][0:D - n1, :],
                                              in_=onr[n1:D, :])

                # -------- output projection emission helpers --------
                # Per i-tile: one st-tagged psum tile [128, 1024] holds both
                # chunks (bank-aligned halves).  i % 3 == 2 uses the mm slots
                # so up to 3 i-tiles are in flight.  Copies alternate
                # DVE / ACT (ACT is idle once the last exp drains).
                op_ps = {}

                def emit_proj_mms(i, ci, trange):
                    isl = slice(i * 128, (i + 1) * 128)
                    off, w = ((0, 512), (512, E - 512))[ci]
                    if trange.start == 0:
                        if i % 4 == 2:
                            if ci == 0:
                                op_ps[i] = {}
                            op_ps[i][ci] = mmps.tile([128, w], F32, tag="mm",
                                                     name="ps_proj")
                        elif i % 4 == 3:
                            if ci == 0:
                                op_ps[i] = {}
                            op_ps[i][ci] = avps.tile([128, w], F32, tag="av",
                                                     bufs=2, name="ps_proj")
                        elif ci == 0:
                            op_ps[i] = stps.tile([128, N], F32, tag="st",
                                                 name="ps_proj")
                    ps = (op_ps[i][ci] if isinstance(op_ps[i], dict)
                          else op_ps[i][:, off:off + w])
                    for t in trange:
                        nc.tensor.matmul(
                            ps, o_all[t][:, isl], wp_sb[t][:, off:off + w],
                            start=(t == 0), stop=(t == ET - 1),
                            skip_group_check=True)

                def emit_proj_out(i, ci):
                    isl = slice(i * 128, (i + 1) * 128)
                    off, w = ((0, 512), (512, E - 512))[ci]
                    osb_t = outp.tile([128, w], F16, tag="out", name="out_sb")
                    ps = (op_ps[i][ci] if isinstance(op_ps[i], dict)
                          else op_ps[i][:, off:off + w])
                    if ci == 0:
                        nc.vector.tensor_copy(osb_t, ps)
                        nc.sync.dma_start(out=out[isl, off:off + w], in_=osb_t)
                    else:
                        nc.scalar.activation(osb_t, ps, COPY)
                        nc.scalar.dma_start(out=out[isl, off:off + w],
                                            in_=osb_t)

                o_sb, rcp, rbc, ex_hist = {}, {}, {}, {}
                for idx in range(4):
                    emit_qk_group(0, idx)
                vq = list(vgroups)
                for h in range(H):
                    # S^T+exp for head h; between j-tiles, emit filler PE
                    # work: v-proj groups (heads 0-1), the next head's q/k
                    # groups, and head h-2's AV/norm (lag 2 keeps the AV off
                    # the exp stream's critical path)
                    o_sb[h] = osb.tile([DA, N], F16, tag="osb", bufs=3,
                                       name=f"osb{h}")
                    ex = []
                    for j in range(NT):
                        exj = exps.tile([128, N], F16, tag=f"ex{j}", name=f"ex{h}_{j}")
                        js = slice(j * 128, (j + 1) * 128)
                        st = stps.tile([128, N], F32, tag="st", name="st_ps")
                        for off in (0, 512):
                            nc.tensor.matmul(
                                st[:, off:off + 512], kT[h][:, js],
                                qT[h][:, off:off + 512],
                                start=True, stop=True)
                        nc.scalar.activation(exj, st, EXP, scale=SCALE)
                        ex.append(exj)
                        if h <= 1:
                            emit_v(*vq.pop(0))
                            if j % 2 == 1:
                                emit_qk_group(h + 1, j // 2)
                        elif h < H - 1:
                            if j % 2 == 0:
                                emit_qk_group(h + 1, j // 2)
                            elif j == 1:
                                emit_av(h - 2, ex_hist[h - 2], 0)
                            elif j == 3:
                                emit_av(h - 2, ex_hist[h - 2], 512)
                            elif j == 5:
                                emit_norm(h - 2, 0)
                            elif j == 7:
                                emit_norm(h - 2, 1)
                        else:
                            # head 7 slot: catch up on heads 5 and 6
                            if j == 1:
                                emit_av(5, ex_hist[5], 0)
                            elif j == 2:
                                emit_av(5, ex_hist[5], 512)
                            elif j == 3:
                                emit_norm(5, 0)
                            elif j == 4:
                                emit_norm(5, 1)
                            elif j == 5:
                                emit_av(6, ex_hist[6], 0)
                            elif j == 6:
                                emit_av(6, ex_hist[6], 512)
                            elif j == 7:
                                emit_norm(6, 0)
                    ex_hist[h] = ex
                    if h >= 2:
                        del ex_hist[h - 2]

                def emit_norm7_half(half):
                    # head 7's normalize is the drain's serial chain: do it
                    # in column halves so the i<4 t=5 finishers (which read
                    # only cols 0..511 of o_all[5]) start after half 0
                    hs = slice(half * 512, half * 512 + 512)
                    if half == 0:
                        rcp[7] = osb.tile([1, N], F16, tag="rcp", name="rcp7")
                        rbc[7] = osb.tile([D, N], F16, tag="rbc", name="rbc7")
                        rcp["onr7"] = osb.tile([D, N], F16, tag="onr",
                                               name="onr7")
                    if half == 0:
                        rcp["dr7"] = dramp.tile([1, N], F16, tag="drcp",
                                                name="drcp7")
                    with nc.allow_low_precision(reason="denom ~1e3"):
                        nc.vector.reciprocal(rcp[7][:, hs], o_sb[7][D:DA, hs])
                    dr7 = rcp["dr7"]
                    nc.sync.dma_start(out=dr7[0:1, hs], in_=rcp[7][0:1, hs])
                    sl = dr7[0:1, hs]
                    bc = bass.AP(tensor=sl.tensor, offset=sl.offset,
                                 ap=[[0, D]] + [list(d) for d in sl.ap[1:]])
                    nc.sync.dma_start(out=rbc[7][:, hs], in_=bc)
                    onr = rcp["onr7"]
                    nc.vector.tensor_mul(onr[:, hs], o_sb[7][0:D, hs],
                                         rbc[7][:, hs])
                    nc.sync.dma_start(out=o_all[5][32:128, hs],
                                      in_=onr[0:D, hs])

                # ---------------- drain + output projection ----------------
                # o_all[0..3] are complete after head 5's repack and
                # o_all[4] after head 6's, so partial t<=4 out-proj groups
                # fill the PE during head 7's exp tail and normalize chain;
                # t=5 finishers follow each repack half.
                # Slot map: i%4 -> st, st, mm, av psum slots.
                emit_norm(6, 1)
                for i in (2, 0):
                    for ci in (0, 1):
                        emit_proj_mms(i, ci, range(0, ET - 2))
                emit_av(7, ex_hist[7], 0)
                emit_norm7_half(0)
                for ci in (0, 1):
                    emit_proj_mms(1, ci, range(0, ET - 2))
                for i in (2, 0, 1):
                    for ci in (0, 1):
                        emit_proj_mms(i, ci, range(ET - 2, ET - 1))
                emit_av(7, ex_hist[7], 512)
                emit_norm7_half(1)
                for ci in (0, 1):
                    emit_proj_mms(3, ci, range(0, ET - 1))
                for i in (2, 0, 1, 3):
                    for ci in (0, 1):
                        emit_proj_mms(i, ci, range(ET - 1, ET))
                    emit_proj_out(i, 0)
                    emit_proj_out(i, 1)
                for i in range(4, NT):
                    for ci in (0, 1):
                        emit_proj_mms(i, ci, range(0, ET))
                    emit_proj_out(i, 0)
                    emit_proj_out(i, 1)

    nc.compile()
    return nc


def prep_weights(Wqkv, bqkv, Wproj, bproj):
    Wr = np.asarray(Wqkv, np.float32).reshape(E, H, D, 3)
    br = np.asarray(bqkv, np.float32).reshape(H, D, 3)
    wq_flat = np.ascontiguousarray(Wr[:, :, :, 0].reshape(E, E)) * WS
    wk_flat = np.ascontiguousarray(Wr[:, :, :, 1].reshape(E, E)) * WS
    # fp8 DoubleRow layout: [pair, 128, 2, cols], contraction k = 256t+128d+p
    wq8 = wq_flat.reshape(KP, 2, 128, E).transpose(0, 2, 1, 3)
    wk8 = wk_flat.reshape(KP, 2, 128, E).transpose(0, 2, 1, 3)
    wv_full = np.zeros((E, H * DA), np.float32)
    for h in range(H):
        wv_full[:, h * DA:h * DA + D] = Wr[:, h, :, 2]
    # host-side output bias: attn rows sum to 1, so attn@(v+bv) = attn@v + bv
    # and (o + bv_cat) @ Wproj + bproj = o @ Wproj + bp_eff
    bv_cat = br[:, :, 2].reshape(E)
    bp_eff = bv_cat @ np.asarray(Wproj, np.float64) + np.asarray(bproj, np.float64)
    return {
        "wq8": np.ascontiguousarray(wq8).astype(NP8),
        "wk8": np.ascontiguousarray(wk8).astype(NP8),
        "qb": np.ascontiguousarray(br[:, :, 0].T) * WS,  # [D, H] fp32
        "wv": wv_full.astype(np.float16),
        "wp": np.asarray(Wproj, np.float32).astype(np.float16),
    }, bp_eff.astype(np.float32)


def make_in_maps(x, Wqkv, bqkv, Wproj, bproj):
    x = np.asarray(x, np.float32)
    shared, bp_eff = prep_weights(Wqkv, bqkv, Wproj, bproj)
    shared["qb"] = shared["qb"].astype(np.float32)
    make_in_maps.bp_eff = bp_eff
    in_maps = []
    for b in range(B):
        xT_b = np.ascontiguousarray(x[b].T)                      # [E, N]
        x8_b = xT_b.reshape(KP, 2, 128, N).transpose(0, 2, 1, 3)  # DR layout
        m = {"xT": xT_b.astype(np.float16),
             "x8": np.ascontiguousarray(x8_b).astype(NP8)}
        m.update(shared)
        in_maps.append(m)
    return in_maps


_prog_cache = []


def kernel(x, Wqkv, bqkv, Wproj, bproj, _run_kwargs=None):
    from concourse.bass_utils import run_bass_kernel_spmd

    in_maps = make_in_maps(x, Wqkv, bqkv, Wproj, bproj)
    if not _prog_cache:
        _prog_cache.append(build_program())
    nc = _prog_cache[0]
    res = run_bass_kernel_spmd(nc, in_maps, core_ids=list(range(B)),
                               **(_run_kwargs or {}))
    out = np.stack([r["out"].astype(np.float32) for r in res.results], axis=0)
    out = out + make_in_maps.bp_eff
    if _run_kwargs:
        kernel.last_result = res
    return out


# revision 4
# speedup vs baseline: 1.0338x; 1.0338x over previous
"""Multi-head attention Bass/Tile kernel for Trainium2, 8 cores data-parallel.

Shapes (hardcoded): x [8, 1024, 768], Wqkv [768, 2304], bqkv [2304],
Wproj [768, 768], bproj [768].  B=8 batches -> one batch per NeuronCore.

v2 dataflow per core (v/out path fp16, q/k path fp8 DoubleRow):
  v     [n, c'] : stationary = xT-tiles fp16, moving = Wv_aug (c' = 8*97,
                  ones col per head for the softmax denominator)
  qT/kT [96, n] : fp8 DoubleRow, K=256 per matmul (3 per 512-chunk);
                  W pre-scaled by 64 host-side (fp8 subnormal range), the
                  64*64 factor is divided out of the exp scale.  q bias is
                  added by DVE tensor_scalar_add on the psum->sbuf copy.
  S^T   [j, i]  : psum [128, 1024] (2 banks), 2 MMs, K=96
  expS^T        : one ACT exp per (h, j) tile with fused *E^-0.5/4096 scale;
                  ACT runs nothing but exp
  o_aug^T [d,i] : stationary = v head cols (96 + ones), moving = expS^T
                  -> row 96 = softmax denominator (colsum)
  normalize     : DVE recip of the denom row; gpsimd partition_broadcast
                  replicates it (no DRAM bounce); DVE fp16 multiply; then
                  SBUF->SBUF DMA repacks the 96-row head block into
                  128-row o_all tiles for a K=128 output projection
  out   [i, e]  : 6 MMs K=128 per (i, chunk); fp16 out, DMA to DRAM;
                  proj+v biases folded on host.
"""

import numpy as np
import ml_dtypes

import concourse.bass as bass
import concourse.bacc as bacc
import concourse.mybir as mybir
import concourse.tile as tile

B, N, E, H = 8, 1024, 768, 8
D = E // H          # 96
DA = D + 1          # 97: head dim + ones column for colsum
NT = N // 128       # 8 token tiles
ET = E // 128       # 6 embedding k-tiles
KP = E // 256       # 3 fp8 DoubleRow k-pair tiles
WS = 64.0           # host-side Wq/Wk scale (fp8 subnormal dodge)
SCALE = float(E) ** -0.5 / (WS * WS)

F16 = mybir.dt.float16
F32 = mybir.dt.float32
F8 = mybir.dt.float8e4
NP8 = ml_dtypes.float8_e4m3
DR = mybir.MatmulPerfMode.DoubleRow
EXP = mybir.ActivationFunctionType.Exp
COPY = mybir.ActivationFunctionType.Copy


def build_program(repeats=1, loop_n=0):
    """loop_n > 0 wraps the body in a hardware For_i loop (timing use)."""
    import contextlib
    nc = bacc.Bacc("TRN2", target_bir_lowering=False)

    xT = nc.dram_tensor("xT", [E, N], F16, kind="ExternalInput")
    x8 = nc.dram_tensor("x8", [KP, 128, 2, N], F8, kind="ExternalInput")
    wq8 = nc.dram_tensor("wq8", [KP, 128, 2, E], F8, kind="ExternalInput")
    wk8 = nc.dram_tensor("wk8", [KP, 128, 2, E], F8, kind="ExternalInput")
    qb = nc.dram_tensor("qb", [D, H], F32, kind="ExternalInput")
    wv = nc.dram_tensor("wv", [E, H * DA], F16, kind="ExternalInput")
    wp = nc.dram_tensor("wp", [E, E], F16, kind="ExternalInput")
    out = nc.dram_tensor("out", [N, E], F16, kind="ExternalOutput")

    with tile.TileContext(nc) as tc:
        with (
            tc.tile_pool(name="persist", bufs=1) as persist,
            tc.tile_pool(name="exps", bufs=3) as exps,
            tc.tile_pool(name="osb", bufs=2) as osb,
            tc.tile_pool(name="outsb", bufs=8) as outp,
            tc.tile_pool(name="mmps", bufs=2, space="PSUM") as mmps,
            tc.tile_pool(name="stps", bufs=2, space="PSUM") as stps,
            tc.tile_pool(name="avps", bufs=2, space="PSUM") as avps,
            tc.tile_pool(name="dramp", bufs=2, space="DRAM") as dramp,
        ):
            loop_cm = (tc.For_i(0, loop_n, 1,
                                hint_engines=tuple(mybir.ALL_ENGINES))
                       if loop_n > 0 else contextlib.nullcontext())
            with loop_cm:
             for _rep in range(repeats):
                # ---------------- load inputs ----------------
                # DMA order = first-use order: x+wv (v phase) first, then the
                # fp8 q/k operands, then wp
                x_sb, wv_sb, x8_sb, wq8_sb, wk8_sb = [], [], [], [], []
                # load order = first-use order: the fp8 q/k operands feed the
                # first PE work (qk(0) -> S^T(0)); x/wv stream behind them for
                # the v-proj filler groups in the head-0/1 slots.  Two HWDGE
                # queues (SP + ACT) run in parallel.
                qb_sb = persist.tile([D, H], F32, tag="qb", name="qb")
                nc.sync.dma_start(out=qb_sb, in_=qb[:, :])
                ones1 = persist.tile([1, D], F16, tag="ones1", name="ones1")
                nc.gpsimd.memset(ones1, 1.0)
                for t in range(KP):
                    xt = persist.tile([128, 2, N], F8, tag=f"x8{t}", name=f"x8{t}")
                    nc.scalar.dma_start(out=xt[:, :, 0:512],
                                        in_=x8[t, :, :, 0:512])
                    x8_sb.append(xt)
                    qt = persist.tile([128, 2, E], F8, tag=f"wq8{t}", name=f"wq8{t}")
                    nc.sync.dma_start(out=qt, in_=wq8[t, :, :, :])
                    wq8_sb.append(qt)
                for t in range(KP):
                    nc.scalar.dma_start(out=x8_sb[t][:, :, 512:N],
                                        in_=x8[t, :, :, 512:N])
                    kt = persist.tile([128, 2, E], F8, tag=f"wk8{t}", name=f"wk8{t}")
                    nc.sync.dma_start(out=kt, in_=wk8[t, :, :, :])
                    wk8_sb.append(kt)
                for k in range(ET):
                    xk = persist.tile([128, N], F16, tag=f"x{k}", name=f"x{k}")
                    nc.scalar.dma_start(out=xk[:, 0:512],
                                        in_=xT[k * 128:(k + 1) * 128, 0:512])
                    x_sb.append(xk)
                    vk = persist.tile([128, H * DA], F16, tag=f"wv{k}", name=f"wv{k}")
                    nc.sync.dma_start(out=vk[:, 0:512],
                                      in_=wv[k * 128:(k + 1) * 128, 0:512])
                    wv_sb.append(vk)
                for k in range(ET):
                    nc.sync.dma_start(out=wv_sb[k][:, 512:H * DA],
                                      in_=wv[k * 128:(k + 1) * 128, 512:H * DA])
                for k in range(ET):
                    nc.scalar.dma_start(out=x_sb[k][:, 512:N],
                                        in_=xT[k * 128:(k + 1) * 128, 512:N])
                wp_sb = []
                for t in range(ET):
                    pt = persist.tile([128, E], F16, tag=f"wp{t}", name=f"wp{t}")
                    eng = nc.sync if t % 2 == 0 else nc.scalar
                    eng.dma_start(out=pt, in_=wp[t * 128:(t + 1) * 128, :])
                    wp_sb.append(pt)

                # ---------------- QKV projections ----------------
                qT = [persist.tile([D, N], F16, tag=f"qT{c}", name=f"qT{c}")
                      for c in range(H)]
                kT = [persist.tile([D, N], F16, tag=f"kT{c}", name=f"kT{c}")
                      for c in range(H)]
                v_sb = [persist.tile([128, H * DA], F16, tag=f"v{n}", name=f"v{n}")
                        for n in range(NT)]

                # v groups are emitted as PE filler inside head-0/1
                # slots (the exp stream leaves the PE half idle there);
                # group order follows DMA arrival
                def emit_v(n, off, w):
                    ns = slice(n * 128, (n + 1) * 128)
                    ps = mmps.tile([128, w], F32, tag="mm", name="ps_v")
                    for k in range(ET):
                        nc.tensor.matmul(
                            ps, x_sb[k][:, ns], wv_sb[k][:, off:off + w],
                            start=(k == 0), stop=(k == ET - 1))
                    nc.vector.tensor_copy(v_sb[n][:, off:off + w], ps)
                    if off == 512:
                        # ones column per head (colsum trick); softmax makes
                        # the k-bias terms cancel, the v-bias folds on host.
                        # Emitted after the second chunk copy of this n-tile
                        # so the copies don't clobber it.
                        nc.gpsimd.memset(
                            v_sb[n].rearrange("p (h a) -> p h a", h=H)[:, :, D],
                            1.0)

                W2 = H * DA - 512
                vgroups = ([(n, 0, 512) for n in range(4)]
                           + [(n, 512, W2) for n in range(4)]
                           + [(n, 0, 512) for n in range(4, NT)]
                           + [(n, 512, W2) for n in range(4, NT)])

                # emit_qk_group(h, idx): one fp8 DoubleRow psum accumulation
                # group (idx 0/1 = q chunks, 2/3 = k chunks)
                def emit_qk_group(h, idx):
                    w8, dst = (wq8_sb, qT[h]) if idx < 2 else (wk8_sb, kT[h])
                    cs = slice(h * D, (h + 1) * D)
                    off = (idx % 2) * 512
                    ps = mmps.tile([D, 512], F32, tag="mm", name="ps_qk")
                    for t in range(KP):
                        nc.tensor.matmul(
                            ps, w8[t][:, :, cs], x8_sb[t][:, :, off:off + 512],
                            start=(t == 0), stop=(t == KP - 1), perf_mode=DR)
                    if idx < 2:
                        nc.vector.tensor_scalar_add(
                            dst[:, off:off + 512], ps, qb_sb[:, h:h + 1])
                    else:
                        nc.vector.tensor_copy(dst[:, off:off + 512], ps)

                o_all = [persist.tile([128, N], F16, tag=f"oa{t}", name=f"oa{t}")
                         for t in range(ET)]

                def emit_av(h, ex, off):
                    hs = slice(h * DA, (h + 1) * DA)
                    av = avps.tile([DA, 512], F32, tag="av", bufs=2,
                                   name="av_ps")
                    for j in range(NT):
                        nc.tensor.matmul(
                            av, v_sb[j][:, hs], ex[j][:, off:off + 512],
                            start=(j == 0), stop=(j == NT - 1))
                    nc.vector.tensor_copy(o_sb[h][:, off:off + 512], av)

                def emit_norm(h, step):
                    if step == 0:
                        # reciprocal of the denominator row, then replicate it
                        # across partitions on the (idle) gpsimd engine
                        rcp[h] = osb.tile([1, N], F16, tag="rcp",
                                          name=f"rcp{h}")
                        with nc.allow_low_precision(reason="denom ~1e3"):
                            nc.vector.reciprocal(rcp[h], o_sb[h][D:DA, :])
                        rbc[h] = osb.tile([D, N], F16, tag="rbc",
                                          name=f"rbc{h}")
                        # replicate across partitions with a DRAM bounce: the
                        # DMA engines are free, and gpsimd's SBUF port is
                        # shared with DVE (measured slower)
                        dr = dramp.tile([1, N], F16, tag="drcp",
                                        name=f"drcp{h}")
                        nc.sync.dma_start(out=dr[0:1, :], in_=rcp[h][0:1, :])
                        bc = bass.AP(
                            tensor=dr.tensor, offset=dr.offset,
                            ap=[[0, D]] + [list(d) for d in dr[0:1, :].ap[1:]])
                        nc.sync.dma_start(out=rbc[h], in_=bc)
                    else:
                        onr = osb.tile([D, N], F16, tag="onr", name=f"onr{h}")
                        nc.vector.tensor_mul(onr, o_sb[h][0:D, :], rbc[h])
                        # repack the 96-row head block into 128-row o_all
                        # tiles (SBUF->SBUF DMA shifts partitions) so the
                        # output projection contracts K=128 per matmul
                        r0 = D * h
                        t0, p0 = r0 // 128, r0 % 128
                        n1 = min(D, 128 - p0)
                        nc.sync.dma_start(out=o_all[t0][p0:p0 + n1, :],
                                          in_=onr[0:n1, :])
                        if n1 < D:
                            nc.sync.dma_start(out=o_all[t0 + 1][0:D - n1, :],
                                              in_=onr[n1:D, :])

                # -------- output projection emission helpers --------
                # Per i-tile: one st-tagged psum tile [128, 1024] holds both
                # chunks (bank-aligned halves).  i % 3 == 2 uses the mm slots
                # so up to 3 i-tiles are in flight.  Copies alternate
                # DVE / ACT (ACT is idle once the last exp drains).
                op_ps = {}

                def emit_proj_mms(i, ci, trange):
                    isl = slice(i * 128, (i + 1) * 128)
                    off, w = ((0, 512), (512, E - 512))[ci]
                    if trange.start == 0:
                        if i % 4 == 2:
                            if ci == 0:
                                op_ps[i] = {}
                            op_ps[i][ci] = mmps.tile([128, w], F32, tag="mm",
                                                     name="ps_proj")
                        elif i % 4 == 3:
                            if ci == 0:
                                op_ps[i] = {}
                            op_ps[i][ci] = avps.tile([128, w], F32, tag="av",
                                                     bufs=2, name="ps_proj")
                        elif ci == 0:
                            op_ps[i] = stps.tile([128, N], F32, tag="st",
                                                 name="ps_proj")
                    ps = (op_ps[i][ci] if isinstance(op_ps[i], dict)
                          else op_ps[i][:, off:off + w])
                    for t in trange:
                        nc.tensor.matmul(
                            ps, o_all[t][:, isl], wp_sb[t][:, off:off + w],
                            start=(t == 0), stop=(t == ET - 1),
                            skip_group_check=True)

                def emit_proj_out(i, ci):
                    isl = slice(i * 128, (i + 1) * 128)
                    off, w = ((0, 512), (512, E - 512))[ci]
                    osb_t = outp.tile([128, w], F16, tag="out", name="out_sb")
                    ps = (op_ps[i][ci] if isinstance(op_ps[i], dict)
                          else op_ps[i][:, off:off + w])
                    if ci == 0:
                        nc.vector.tensor_copy(osb_t, ps)
                        nc.sync.dma_start(out=out[isl, off:off + w], in_=osb_t)
                    else:
                        nc.scalar.activation(osb_t, ps, COPY)
                        nc.scalar.dma_start(out=out[isl, off:off + w],
                                            in_=osb_t)

                o_sb, rcp, rbc, ex_hist = {}, {}, {}, {}
                for idx in range(4):
                    emit_qk_group(0, idx)
                vq = list(vgroups)
                for h in range(H):
                    # S^T+exp for head h; between j-tiles, emit filler PE
                    # work: v-proj groups (heads 0-1), the next head's q/k
                    # groups, and head h-2's AV/norm (lag 2 keeps the AV off
                    # the exp stream's critical path)
                    o_sb[h] = osb.tile([DA, N], F16, tag="osb", bufs=3,
                                       name=f"osb{h}")
                    ex = []
                    for j in range(NT):
                        exj = exps.tile([128, N], F16, tag=f"ex{j}", name=f"ex{h}_{j}")
                        js = slice(j * 128, (j + 1) * 128)
                        st = stps.tile([128, N], F32, tag="st", name="st_ps")
                        for off in (0, 512):
                            nc.tensor.matmul(
                                st[:, off:off + 512], kT[h][:, js],
                                qT[h][:, off:off + 512],
                                start=True, stop=True)
                        nc.scalar.activation(exj, st, EXP, scale=SCALE)
                        ex.append(exj)
                        if h <= 1:
                            emit_v(*vq.pop(0))
                            if j % 2 == 1:
                                emit_qk_group(h + 1, j // 2)
                        elif h < H - 1:
                            if j % 2 == 0:
                                emit_qk_group(h + 1, j // 2)
                            elif j == 1:
                                emit_av(h - 2, ex_hist[h - 2], 0)
                            elif j == 3:
                                emit_av(h - 2, ex_hist[h - 2], 512)
                            elif j == 5:
                                emit_norm(h - 2, 0)
                            elif j == 7:
                                emit_norm(h - 2, 1)
                        else:
                            # head 7 slot: catch up on heads 5 and 6.  AV
                            # groups go on odd j only so the in-order PE
                            # queue reaches S^T(7,7) (which gates the whole
                            # drain via exp(7,7)) with minimal filler ahead
                            if j == 1:
                                emit_av(5, ex_hist[5], 0)
                            elif j == 3:
                                emit_av(5, ex_hist[5], 512)
                            elif j == 4:
                                emit_norm(5, 0)
                            elif j == 5:
                                emit_av(6, ex_hist[6], 0)
                            elif j == 6:
                                emit_norm(5, 1)
                            elif j == 7:
                                emit_av(6, ex_hist[6], 512)
                                emit_norm(6, 0)
                    ex_hist[h] = ex
                    if h >= 2:
                        del ex_hist[h - 2]

                def emit_norm7_half(half):
                    # head 7's normalize is the drain's serial chain: do it
                    # in column halves so the i<4 t=5 finishers (which read
                    # only cols 0..511 of o_all[5]) start after half 0
                    hs = slice(half * 512, half * 512 + 512)
                    if half == 0:
                        rcp[7] = osb.tile([1, N], F16, tag="rcp", name="rcp7")
                        rcp["onr7"] = osb.tile([D, N], F16, tag="onr",
                                               name="onr7")
                    if half == 0:
                        rcp["bc7"] = stps.tile([128, N], F32, tag="st",
                                               name="rbc7ps")
                    with nc.allow_low_precision(reason="denom ~1e3"):
                        nc.vector.reciprocal(rcp[7][:, hs], o_sb[7][D:DA, hs])
                    # replicate across partitions with a K=1 ones matmul on
                    # the (idle-at-drain) PE: deterministic, tracked, and
                    # ~2us faster than the DRAM bounce
                    bc7 = rcp["bc7"]
                    nc.tensor.matmul(bc7[0:D, hs], ones1[0:1, 0:D],
                                     rcp[7][0:1, hs], start=True, stop=True)
                    onr = rcp["onr7"]
                    nc.vector.tensor_mul(onr[:, hs], o_sb[7][0:D, hs],
                                         bc7[0:D, hs])
                    nc.sync.dma_start(out=o_all[5][32:128, hs],
                                      in_=onr[0:D, hs])

                # ---------------- drain + output projection ----------------
                # o_all[0..3] are complete after head 5's repack and
                # o_all[4] after head 6's, so partial t<=4 out-proj groups
                # fill the PE during head 7's exp tail and normalize chain;
                # t=5 finishers follow each repack half.
                # Slot map: i%4 -> st, st, mm, av psum slots.
                emit_norm(6, 1)
                emit_av(7, ex_hist[7], 0)
                emit_norm7_half(0)
                emit_av(7, ex_hist[7], 512)
                emit_norm7_half(1)
                for i in (2, 0):
                    for ci in (0, 1):
                        emit_proj_mms(i, ci, range(0, ET - 2))
                for ci in (0, 1):
                    emit_proj_mms(1, ci, range(0, ET - 2))
                for i in (2, 0, 1):
                    for ci in (0, 1):
                        emit_proj_mms(i, ci, range(ET - 2, ET - 1))
                for ci in (0, 1):
                    emit_proj_mms(3, ci, range(0, ET - 1))
                for i in (2, 0, 1, 3):
                    for ci in (0, 1):
                        emit_proj_mms(i, ci, range(ET - 1, ET))
                    emit_proj_out(i, 0)
                    emit_proj_out(i, 1)
                for i in range(4, NT):
                    for ci in (0, 1):
                        emit_proj_mms(i, ci, range(0, ET))
                    emit_proj_out(i, 0)
                    emit_proj_out(i, 1)

    nc.compile()
    return nc


def prep_weights(Wqkv, bqkv, Wproj, bproj):
    Wr = np.asarray(Wqkv, np.float32).reshape(E, H, D, 3)
    br = np.asarray(bqkv, np.float32).reshape(H, D, 3)
    wq_flat = np.ascontiguousarray(Wr[:, :, :, 0].reshape(E, E)) * WS
    wk_flat = np.ascontiguousarray(Wr[:, :, :, 1].reshape(E, E)) * WS
    # fp8 DoubleRow layout: [pair, 128, 2, cols], contraction k = 256t+128d+p
    wq8 = wq_flat.reshape(KP, 2, 128, E).transpose(0, 2, 1, 3)
    wk8 = wk_flat.reshape(KP, 2, 128, E).transpose(0, 2, 1, 3)
    wv_full = np.zeros((E, H * DA), np.float32)
    for h in range(H):
        wv_full[:, h * DA:h * DA + D] = Wr[:, h, :, 2]
    # host-side output bias: attn rows sum to 1, so attn@(v+bv) = attn@v + bv
    # and (o + bv_cat) @ Wproj + bproj = o @ Wproj + bp_eff
    bv_cat = br[:, :, 2].reshape(E)
    bp_eff = bv_cat @ np.asarray(Wproj, np.float64) + np.asarray(bproj, np.float64)
    return {
        "wq8": np.ascontiguousarray(wq8).astype(NP8),
        "wk8": np.ascontiguousarray(wk8).astype(NP8),
        "qb": np.ascontiguousarray(br[:, :, 0].T) * WS,  # [D, H] fp32
        "wv": wv_full.astype(np.float16),
        "wp": np.asarray(Wproj, np.float32).astype(np.float16),
    }, bp_eff.astype(np.float32)


def make_in_maps(x, Wqkv, bqkv, Wproj, bproj):
    x = np.asarray(x, np.float32)
    shared, bp_eff = prep_weights(Wqkv, bqkv, Wproj, bproj)
    shared["qb"] = shared["qb"].astype(np.float32)
    make_in_maps.bp_eff = bp_eff
    in_maps = []
    for b in range(B):
        xT_b = np.ascontiguousarray(x[b].T)                      # [E, N]
        x8_b = xT_b.reshape(KP, 2, 128, N).transpose(0, 2, 1, 3)  # DR layout
        m = {"xT": xT_b.astype(np.float16),
             "x8": np.ascontiguousarray(x8_b).astype(NP8)}
        m.update(shared)
        in_maps.append(m)
    return in_maps


_prog_cache = []


def kernel(x, Wqkv, bqkv, Wproj, bproj, _run_kwargs=None):
    from concourse.bass_utils import run_bass_kernel_spmd

    in_maps = make_in_maps(x, Wqkv, bqkv, Wproj, bproj)
    if not _prog_cache:
        _prog_cache.append(build_program())
    nc = _prog_cache[0]
    res = run_bass_kernel_spmd(nc, in_maps, core_ids=list(range(B)),
                               **(_run_kwargs or {}))
    out = np.stack([r["out"].astype(np.float32) for r in res.results], axis=0)
    out = out + make_in_maps.bp_eff
    if _run_kwargs:
        kernel.last_result = res
    return out
